# revision 18
# baseline (speedup 1.0000x reference)
"""Block-sparse DSD matmul  y = x @ W^T  on 8 TRN2 NeuronCores.

x: [2048, 4096] f32, W given as 2048 sparse 32x32 blocks at (rows, cols)
block coordinates in a 128x128 block grid. y: [2048, 4096] f32.

Strategy (batch-parallel SPMD, identical program on 8 cores):
  - Shard batch 8 ways (256 rows/core); the sparse structure is identical
    on every core so one SPMD program works with per-core x shards.
  - bf16 x and W (f32 PSUM accumulation) — one PE pass per matmul and
    half the HBM traffic; y returned as bf16 and widened on host.
  - Compute y^T tiles on-chip: for block (r, c):
        y^T[32r:32r+32, :] += W_blk @ x^T[32c:32c+32, :]
    As a PE matmul: out = lhsT.T @ rhs with lhsT = W_blk^T (stationary,
    32x32), rhs = x^T chunk [32, 256], tile_position picks the 32x32 PE
    subarray: row group a = c%4 (SBUF strip), col group b = output strip.
  - Rows are sorted by nnz count and grouped 4-at-a-time (similar counts
    together) into 32 groups; each group accumulates directly into its
    own PSUM bank ([128, 256] tile, bank-granular pool) via per-strip
    has_written chains — no cross-lane tree-add, one evacuation copy.
  - Matmuls are emitted in "waves" of <=4 with pairwise-distinct col
    groups (and lanes), letting up to 4 subarray matmul streams overlap.
    A greedy scheduler packs slots into waves across a sliding window of
    active groups and prefers early x chunks in early waves.
"""

import numpy as np

# toggles used by test.py only; harness uses defaults
_RUN = {"trace": False, "trace_cores": [0], "last": None}

B, K, OUT, BLK, NNZ = 2048, 4096, 4096, 32, 2048
NCORES = 8
BC = B // NCORES          # 256 batch rows per core
NT = K // 128             # 32 x^T partition-tiles
NRB = OUT // BLK          # 128 row blocks
NG = NRB // 4             # 32 groups of 4 row blocks
NXCH = 8                  # x^T DMA chunks
XC = NT // NXCH           # x tiles per chunk
NWCH = 8                  # weight DMA chunks


def _build_schedule(w, rows, cols):
    """Sorted-count grouping + greedy wave schedule + packed weights.

    Region (g, b) accumulates row order[4g+b] on PSUM bank g's strip b,
    always from PE subarray (32b, 32b) — a PSUM region must only ever be
    written from one tile_position row group (HW constraint).  The rhs
    x^T strip (c%4) is independent of the tile position; waves prefer
    pairwise-distinct strips to share the 128-partition moving port.
    """
    import ml_dtypes

    cnt = np.bincount(rows, minlength=NRB)
    order = np.argsort(-cnt, kind="stable")
    place = {int(r): (k // 4, k % 4) for k, r in enumerate(order)}

    # slots[g][b] = [t, s, n]: blocks of row order[4g+b]; s = rhs strip.
    # n == -1 is a zero-weight dummy for otherwise-untouched PSUM regions.
    slots = [[[] for _ in range(4)] for _ in range(NG)]
    for n in range(NNZ):
        g, b = place[int(rows[n])]
        slots[g][b].append([int(cols[n]) // 4, int(cols[n]) % 4, n])
    for r in range(NRB):
        if cnt[r] == 0:
            g, b = place[r]
            slots[g][b].append([0, 0, -1])
    for g in range(NG):
        for b in range(4):
            slots[g][b].sort(key=lambda s: s[0])

    n_gb = np.array([[len(slots[g][b]) for b in range(4)] for g in range(NG)])

    MAX_ACTIVE = 7
    # x chunk-rotation tiles arrive by DMA at ~LOAD_NS apiece (issue order
    # c-major, k-minor, after the first weight chunk); a slot is eligible
    # once its tile has landed by the wave's estimated start time.
    LOAD_NS, W0_NS, WAVE_MM_NS = 800.0, 1600.0, 35.5
    eta = {(ci, k): W0_NS + LOAD_NS * (4 * ci + k + 1)
           for ci in range(NXCH) for k in range(4)}
    remaining = [[list(l) for l in gl] for gl in slots]
    done_g = [all(not l for l in gl) for gl in remaining]
    seen_gb = np.zeros((NG, 4), dtype=np.int64)
    waves = []
    scheduled = 0
    total = int(n_gb.sum())
    tnow = W0_NS + LOAD_NS
    while scheduled < total:
        used_a = set()
        wave = []
        acts = [g for g in range(NG) if not done_g[g]][:MAX_ACTIVE]
        for b in range(4):
            pick = None
            for g in acts:
                A = (b + g) % 4
                if A in used_a:
                    continue
                for i, (t, s, n) in enumerate(remaining[g][b]):
                    if eta[(t // XC, (A - s) % 4)] > tnow:
                        continue
                    pick = (g, i, t, s, n)
                    break
                if pick:
                    break
            if pick is None:
                continue
            g, i, t, s, n = pick
            remaining[g][b].pop(i)
            used_a.add((b + g) % 4)
            st = seen_gb[g, b] == 0
            sp = seen_gb[g, b] == n_gb[g, b] - 1
            seen_gb[g, b] += 1
            wave.append((s, g, b, t, n, bool(st), bool(sp)))
            scheduled += 1
            if all(not l for l in remaining[g]):
                done_g[g] = True
        if wave:
            waves.append(wave)
            tnow += WAVE_MM_NS * len(wave)
        else:
            # starved: PE would idle until the next tile lands
            nxt = min((eta[(t // XC, ((b + g) % 4 - s) % 4)]
                       for g in range(NG) if not done_g[g]
                       for b in range(4)
                       for (t, s, n) in remaining[g][b]), default=tnow)
            tnow = max(tnow + WAVE_MM_NS, nxt)
            waves.append(wave)

    W = len(waves)
    WCH = -(-W // NWCH)
    Wpad = WCH * NWCH
    wpk = np.zeros((128, Wpad * BLK), dtype=ml_dtypes.bfloat16)
    for wv, wave in enumerate(waves):
        for s, g, b, t, n, st, sp in wave:
            if n >= 0:
                A = (b + g) % 4
                wpk[32 * A:32 * A + 32, wv * BLK:(wv + 1) * BLK] = \
                    np.ascontiguousarray(w[n].T).astype(ml_dtypes.bfloat16)

    # wave index after which each group is fully accumulated
    evac = [-1] * NG
    for wv, wave in enumerate(waves):
        for s, g, b, t, n, st, sp in wave:
            evac[g] = max(evac[g], wv)
    return waves, WCH, Wpad, wpk, order, evac


def _build_module(waves, WCH, Wpad, evac):
    import concourse.bacc as bacc
    import concourse.tile as tile
    import concourse.mybir as mybir
    from contextlib import ExitStack

    f32 = mybir.dt.float32
    bf16 = mybir.dt.bfloat16
    nc = bacc.Bacc()
    xt_d = nc.declare_dram_parameter("xt", [128, 4, NT * BC], bf16,
                                     isOutput=False)
    wp_d = nc.declare_dram_parameter("wpk", [128, Wpad * BLK], bf16,
                                     isOutput=False)
    yt_d = nc.declare_dram_parameter("yt", [128, NG, BC], f32, isOutput=True)

    with tile.TileContext(nc) as tc, ExitStack() as ctx:
        xp = ctx.enter_context(tc.tile_pool(name="x", bufs=1))
        wpool = ctx.enter_context(tc.tile_pool(name="w", bufs=3))
        pp = ctx.enter_context(tc.tile_pool(name="ps", bufs=8, space="PSUM"))
        yp = ctx.enter_context(tc.tile_pool(name="y", bufs=4))

        wtiles = {}

        def load_w(c):
            wsb = wpool.tile([128, WCH * BLK], bf16, tag="w", name=f"w{c}")
            nc.sync.dma_start(
                wsb[:], wp_d[:, c * WCH * BLK:(c + 1) * WCH * BLK])
            wtiles[c] = wsb

        # xrot[k][ci]: x^T chunk pre-rotated on host so that original
        # strip s sits at partition strip (s + k) % 4.  Any column can feed
        # any PE row group while weights/fmap/tile_position stay aligned.
        # DMA issue costs ~600ns of sequencer time apiece, so issues are
        # split across the two HWDGE sequencers (sync + scalar).
        xrot = [[] for _ in range(4)]

        def load_x(ci):
            lo, hi = ci * XC * BC, (ci + 1) * XC * BC
            for k in range(4):
                xc = xp.tile([128, XC * BC], bf16, tag=f"x{k}_{ci}",
                             name=f"x{k}_{ci}")
                eng = nc.sync if k < 2 else nc.scalar
                eng.dma_start(xc[:], xt_d[:, k, lo:hi])
                xrot[k].append(xc)

        load_w(0)
        for ci in range(NXCH):
            load_x(ci)
        load_w(1)

        ptiles = {}
        for wv, wave in enumerate(waves):
            c = wv // WCH
            if wv == c * WCH and c + 2 < NWCH and (c + 2) * WCH < len(waves):
                load_w(c + 2)
            wsb = wtiles[c]
            for s, g, b, t, n, st, sp in wave:
                if g not in ptiles:
                    ptiles[g] = pp.tile([128, BC], f32, tag="ps",
                                        name=f"ps{g}")
                A = (b + g) % 4
                nc.tensor.matmul(
                    ptiles[g][32 * b:32 * b + 32, :],
                    lhsT=wsb[32 * A:32 * A + 32,
                             (wv - c * WCH) * BLK:(wv - c * WCH + 1) * BLK],
                    rhs=xrot[(A - s) % 4][t // XC][
                        32 * A:32 * A + 32,
                        (t % XC) * BC:(t % XC + 1) * BC],
                    start=st, stop=sp, skip_group_check=True,
                    tile_position=(32 * A, 32 * b),
                )
            for g in range(NG):
                if evac[g] == wv:
                    ps = ptiles.pop(g)
                    ysb = yp.tile([128, BC], f32, tag="y", name=f"y{g}")
                    nc.vector.tensor_copy(ysb[:], ps[:])
                    nc.scalar.dma_start(yt_d[:, g, :], ysb[:])

    nc.compile()
    return nc


def kernel(x, w, rows, cols, out_blocks=None):
    import ml_dtypes
    from concourse.bass_utils import run_bass_kernel_spmd

    x = np.asarray(x, dtype=np.float32)
    w = np.asarray(w, dtype=np.float32)
    rows = np.asarray(rows).astype(np.int64)
    cols = np.asarray(cols).astype(np.int64)

    waves, WCH, Wpad, wpk, order, evac = _build_schedule(w, rows, cols)
    nc = _build_module(waves, WCH, Wpad, evac)

    # x^T, per-core partition-major: xarr[core, p, t*BC + j] = x[BC*core + j, 128*t + p]
    xarr = np.ascontiguousarray(
        x.reshape(NCORES, BC, NT, 128).transpose(0, 3, 2, 1)
    ).reshape(NCORES, 128, NT * BC).astype(ml_dtypes.bfloat16)
    # 4 partition rotations: xrot[core, p, k, :] = xarr[core, (p - 32k) % 128, :]
    xrot = np.stack([np.roll(xarr, 32 * k, axis=1) for k in range(4)],
                    axis=2)

    in_maps = [{"xt": xrot[i], "wpk": wpk} for i in range(NCORES)]
    res = run_bass_kernel_spmd(
        nc, in_maps, list(range(NCORES)),
        trace=_RUN["trace"], trace_cores=_RUN["trace_cores"],
    )
    _RUN["last"] = res

    # feature index of flat position (k=4g+b, i): 32*order[k] + i
    feat = (32 * order[:, None] + np.arange(32)[None, :]).ravel()

    y = np.empty((B, OUT), dtype=np.float32)
    for i in range(NCORES):
        ytp = np.asarray(res.results[i]["yt"]).astype(np.float32)
        # [128, NG, 256]: partition 32b+i, group g, batch j -> k=4g+b
        v = ytp.reshape(4, 32, NG, BC).transpose(2, 0, 1, 3)
        yT = np.empty((OUT, BC), dtype=np.float32)
        yT[feat] = v.reshape(OUT, BC)
        y[BC * i:BC * (i + 1), :] = yT.T
    return y


# revision 19
# speedup vs baseline: 1.0439x; 1.0439x over previous
"""Block-sparse DSD matmul  y = x @ W^T  on 8 TRN2 NeuronCores.

x: [2048, 4096] f32, W given as 2048 sparse 32x32 blocks at (rows, cols)
block coordinates in a 128x128 block grid. y: [2048, 4096] f32.

Strategy (batch-parallel SPMD, identical program on 8 cores):
  - Shard batch 8 ways (256 rows/core); the sparse structure is identical
    on every core so one SPMD program works with per-core x shards.
  - bf16 x and W (f32 PSUM accumulation) — one PE pass per matmul and
    half the HBM traffic; y returned as bf16 and widened on host.
  - Compute y^T tiles on-chip: for block (r, c):
        y^T[32r:32r+32, :] += W_blk @ x^T[32c:32c+32, :]
    As a PE matmul: out = lhsT.T @ rhs with lhsT = W_blk^T (stationary,
    32x32), rhs = x^T chunk [32, 256], tile_position picks the 32x32 PE
    subarray: row group a = c%4 (SBUF strip), col group b = output strip.
  - Rows are sorted by nnz count and grouped 4-at-a-time (similar counts
    together) into 32 groups; each group accumulates directly into its
    own PSUM bank ([128, 256] tile, bank-granular pool) via per-strip
    has_written chains — no cross-lane tree-add, one evacuation copy.
  - Matmuls are emitted in "waves" of <=4 with pairwise-distinct col
    groups (and lanes), letting up to 4 subarray matmul streams overlap.
    A greedy scheduler packs slots into waves across a sliding window of
    active groups and prefers early x chunks in early waves.
"""

import numpy as np

# toggles used by test.py only; harness uses defaults
_RUN = {"trace": False, "trace_cores": [0], "last": None}

B, K, OUT, BLK, NNZ = 2048, 4096, 4096, 32, 2048
NCORES = 8
BC = B // NCORES          # 256 batch rows per core
NT = K // 128             # 32 x^T partition-tiles
NRB = OUT // BLK          # 128 row blocks
NG = NRB // 4             # 32 groups of 4 row blocks
NXCH = 8                  # x^T DMA chunks
XC = NT // NXCH           # x tiles per chunk
NWCH = 8                  # weight DMA chunks


def _build_schedule(w, rows, cols):
    """Sorted-count grouping + greedy wave schedule + packed weights.

    Region (g, b) accumulates row order[4g+b] on PSUM bank g's strip b,
    always from PE subarray (32b, 32b) — a PSUM region must only ever be
    written from one tile_position row group (HW constraint).  The rhs
    x^T strip (c%4) is independent of the tile position; waves prefer
    pairwise-distinct strips to share the 128-partition moving port.
    """
    import ml_dtypes

    cnt = np.bincount(rows, minlength=NRB)
    order = np.argsort(-cnt, kind="stable")
    place = {int(r): (k // 4, k % 4) for k, r in enumerate(order)}

    # slots[g][b] = [t, s, n]: blocks of row order[4g+b]; s = rhs strip.
    # n == -1 is a zero-weight dummy for otherwise-untouched PSUM regions.
    slots = [[[] for _ in range(4)] for _ in range(NG)]
    for n in range(NNZ):
        g, b = place[int(rows[n])]
        slots[g][b].append([int(cols[n]) // 4, int(cols[n]) % 4, n])
    for r in range(NRB):
        if cnt[r] == 0:
            g, b = place[r]
            slots[g][b].append([0, 0, -1])
    for g in range(NG):
        for b in range(4):
            slots[g][b].sort(key=lambda s: s[0])

    n_gb = np.array([[len(slots[g][b]) for b in range(4)] for g in range(NG)])

    MAX_ACTIVE = 7
    remaining = [[list(l) for l in gl] for gl in slots]
    done_g = [all(not l for l in gl) for gl in remaining]
    seen_gb = np.zeros((NG, 4), dtype=np.int64)
    waves = []
    scheduled = 0
    total = int(n_gb.sum())
    while scheduled < total:
        wv = len(waves)
        maxchunk = 1 + wv // 22
        used_a = set()
        wave = []
        acts = [g for g in range(NG) if not done_g[g]][:MAX_ACTIVE]
        for b in range(4):
            pick = None
            for g in acts:
                if (b + g) % 4 in used_a:
                    continue
                for i, (t, s, n) in enumerate(remaining[g][b]):
                    if t // XC >= maxchunk:
                        continue
                    pick = (g, i, t, s, n)
                    break
                if pick:
                    break
            if pick is None:
                continue
            g, i, t, s, n = pick
            remaining[g][b].pop(i)
            used_a.add((b + g) % 4)
            st = seen_gb[g, b] == 0
            sp = seen_gb[g, b] == n_gb[g, b] - 1
            seen_gb[g, b] += 1
            wave.append((s, g, b, t, n, bool(st), bool(sp)))
            scheduled += 1
            if all(not l for l in remaining[g]):
                done_g[g] = True
        waves.append(wave)

    W = len(waves)
    WCH = -(-W // NWCH)
    Wpad = WCH * NWCH
    wpk = np.zeros((128, Wpad * BLK), dtype=ml_dtypes.bfloat16)
    for wv, wave in enumerate(waves):
        for s, g, b, t, n, st, sp in wave:
            if n >= 0:
                A = (b + g) % 4
                wpk[32 * A:32 * A + 32, wv * BLK:(wv + 1) * BLK] = \
                    np.ascontiguousarray(w[n].T).astype(ml_dtypes.bfloat16)

    # wave index after which each group is fully accumulated
    evac = [-1] * NG
    for wv, wave in enumerate(waves):
        for s, g, b, t, n, st, sp in wave:
            evac[g] = max(evac[g], wv)
    return waves, WCH, Wpad, wpk, order, evac


def _build_module(waves, WCH, Wpad, evac):
    import concourse.bacc as bacc
    import concourse.tile as tile
    import concourse.mybir as mybir
    from contextlib import ExitStack

    f32 = mybir.dt.float32
    bf16 = mybir.dt.bfloat16
    nc = bacc.Bacc()
    xt_d = nc.declare_dram_parameter("xt", [128, 4, NT * BC], bf16,
                                     isOutput=False)
    wp_d = nc.declare_dram_parameter("wpk", [128, Wpad * BLK], bf16,
                                     isOutput=False)
    yt_d = nc.declare_dram_parameter("yt", [128, NG, BC], bf16, isOutput=True)

    with tile.TileContext(nc) as tc, ExitStack() as ctx:
        xp = ctx.enter_context(tc.tile_pool(name="x", bufs=1))
        wpool = ctx.enter_context(tc.tile_pool(name="w", bufs=3))
        pp = ctx.enter_context(tc.tile_pool(name="ps", bufs=8, space="PSUM"))
        yp = ctx.enter_context(tc.tile_pool(name="y", bufs=4))

        wtiles = {}

        def load_w(c):
            wsb = wpool.tile([128, WCH * BLK], bf16, tag="w", name=f"w{c}")
            nc.sync.dma_start(
                wsb[:], wp_d[:, c * WCH * BLK:(c + 1) * WCH * BLK])
            wtiles[c] = wsb

        # xrot[k][ci]: x^T chunk pre-rotated on host so that original
        # strip s sits at partition strip (s + k) % 4.  Any column can feed
        # any PE row group while weights/fmap/tile_position stay aligned.
        # DMA issue costs ~600ns of sequencer time apiece, so issues are
        # split across the two HWDGE sequencers (sync + scalar).
        xrot = [[] for _ in range(4)]

        def load_x(ci):
            lo, hi = ci * XC * BC, (ci + 1) * XC * BC
            for k in range(4):
                xc = xp.tile([128, XC * BC], bf16, tag=f"x{k}_{ci}",
                             name=f"x{k}_{ci}")
                eng = nc.sync if k < 2 else nc.scalar
                eng.dma_start(xc[:], xt_d[:, k, lo:hi])
                xrot[k].append(xc)

        load_w(0)
        for ci in range(NXCH):
            load_x(ci)
        load_w(1)

        ptiles = {}
        for wv, wave in enumerate(waves):
            c = wv // WCH
            if wv == c * WCH and c + 2 < NWCH and (c + 2) * WCH < len(waves):
                load_w(c + 2)
            wsb = wtiles[c]
            for s, g, b, t, n, st, sp in wave:
                if g not in ptiles:
                    ptiles[g] = pp.tile([128, BC], f32, tag="ps",
                                        name=f"ps{g}")
                A = (b + g) % 4
                nc.tensor.matmul(
                    ptiles[g][32 * b:32 * b + 32, :],
                    lhsT=wsb[32 * A:32 * A + 32,
                             (wv - c * WCH) * BLK:(wv - c * WCH + 1) * BLK],
                    rhs=xrot[(A - s) % 4][t // XC][
                        32 * A:32 * A + 32,
                        (t % XC) * BC:(t % XC + 1) * BC],
                    start=st, stop=sp, skip_group_check=True,
                    tile_position=(32 * A, 32 * b),
                )
            for g in range(NG):
                if evac[g] == wv:
                    ps = ptiles.pop(g)
                    ysb = yp.tile([128, BC], bf16, tag="y", name=f"y{g}")
                    nc.vector.tensor_copy(ysb[:], ps[:])
                    nc.scalar.dma_start(yt_d[:, g, :], ysb[:])

    nc.compile()
    return nc


def kernel(x, w, rows, cols, out_blocks=None):
    import ml_dtypes
    from concourse.bass_utils import run_bass_kernel_spmd

    x = np.asarray(x, dtype=np.float32)
    w = np.asarray(w, dtype=np.float32)
    rows = np.asarray(rows).astype(np.int64)
    cols = np.asarray(cols).astype(np.int64)

    waves, WCH, Wpad, wpk, order, evac = _build_schedule(w, rows, cols)
    nc = _build_module(waves, WCH, Wpad, evac)

    # x^T, per-core partition-major: xarr[core, p, t*BC + j] = x[BC*core + j, 128*t + p]
    xarr = np.ascontiguousarray(
        x.reshape(NCORES, BC, NT, 128).transpose(0, 3, 2, 1)
    ).reshape(NCORES, 128, NT * BC).astype(ml_dtypes.bfloat16)
    # 4 partition rotations: xrot[core, p, k, :] = xarr[core, (p - 32k) % 128, :]
    xrot = np.stack([np.roll(xarr, 32 * k, axis=1) for k in range(4)],
                    axis=2)

    in_maps = [{"xt": xrot[i], "wpk": wpk} for i in range(NCORES)]
    res = run_bass_kernel_spmd(
        nc, in_maps, list(range(NCORES)),
        trace=_RUN["trace"], trace_cores=_RUN["trace_cores"],
    )
    _RUN["last"] = res

    # feature index of flat position (k=4g+b, i): 32*order[k] + i
    feat = (32 * order[:, None] + np.arange(32)[None, :]).ravel()

    y = np.empty((B, OUT), dtype=np.float32)
    for i in range(NCORES):
        ytp = np.asarray(res.results[i]["yt"]).astype(np.float32)
        # [128, NG, 256]: partition 32b+i, group g, batch j -> k=4g+b
        v = ytp.reshape(4, 32, NG, BC).transpose(2, 0, 1, 3)
        yT = np.empty((OUT, BC), dtype=np.float32)
        yT[feat] = v.reshape(OUT, BC)
        y[BC * i:BC * (i + 1), :] = yT.T
    return y


# revision 22
# speedup vs baseline: 1.0855x; 1.0399x over previous
"""Block-sparse DSD matmul  y = x @ W^T  on 8 TRN2 NeuronCores.

x: [2048, 4096] f32, W given as 2048 sparse 32x32 blocks at (rows, cols)
block coordinates in a 128x128 block grid. y: [2048, 4096] f32.

Strategy (batch-parallel SPMD, identical program on 8 cores):
  - Shard batch 8 ways (256 rows/core); the sparse structure is identical
    on every core so one SPMD program works with per-core x shards.
  - Compute y^T tiles on-chip: for block (r, c):
        y^T[32r:32r+32, :] += W_blk @ x^T[32c:32c+32, :]
    As a PE matmul: out = lhsT.T @ rhs with lhsT = W_blk^T (stationary,
    32x32), rhs = x^T chunk [32, 256].
  - 16-way 32x32 PE subarray tiling: lane a = c%4 picks the SBUF
    partition strip (and PE row group); row-blocks are packed 4 to a
    "group", strip b in the group picks the PSUM partition strip (PE col
    group).  Each lane accumulates into its own PSUM bank; a DVE tree-add
    of the 4 lane banks produces the group's y^T tile [128, 256].
  - Host: pre-transposes x into partition-major per-core layout, packs
    transposed weight blocks into a lane-major array, assembles y.
"""

import numpy as np

# toggles used by test.py only; harness uses defaults
_RUN = {"trace": False, "trace_cores": [0], "last": None}

B, K, OUT, BLK, NNZ = 2048, 4096, 4096, 32, 2048
NCORES = 8
BC = B // NCORES          # 256 batch rows per core
NT = K // 128             # 32 x^T partition-tiles
NRB = OUT // BLK          # 128 row blocks
NG = NRB // 4             # 32 groups of 4 row blocks


def _build_schedule(w, rows, cols):
    """Group assignment + per-(group, lane) slot schedule + packed weights."""
    cnt = np.bincount(rows, minlength=NRB)
    order = np.argsort(-cnt, kind="stable")
    rmap = np.empty((NG, 4), dtype=np.int64)
    for rank, r in enumerate(order):
        rnd, pos = rank // NG, rank % NG
        g = pos if rnd % 2 == 0 else NG - 1 - pos
        rmap[g, rnd] = r

    gb_of_row = {}
    for g in range(NG):
        for b in range(4):
            gb_of_row[int(rmap[g, b])] = (g, b)

    cells = [[[[] for _ in range(4)] for _ in range(4)] for _ in range(NG)]
    for n in range(NNZ):
        g, b = gb_of_row[int(rows[n])]
        cells[g][int(cols[n]) % 4][b].append(n)

    # prog[g][a] = list of slots (t, b, start, stop, wT[32,32]), sorted by
    # x-tile index t so matmuls become eligible as x chunks stream in.
    prog = []
    for g in range(NG):
        lanes = []
        for a in range(4):
            raw = []
            for b in range(4):
                cl = cells[g][a][b]
                if not cl:
                    raw.append((0, b, np.zeros((BLK, BLK), np.float32)))
                for n in cl:
                    raw.append((int(cols[n]) // 4, b,
                                np.ascontiguousarray(w[n].T)))
            raw.sort(key=lambda s: s[0])
            first = {}
            last = {}
            for i, (_, b, _) in enumerate(raw):
                first.setdefault(b, i)
                last[b] = i
            slots = [(t, b, i == first[b], i == last[b], wt)
                     for i, (t, b, wt) in enumerate(raw)]
            lanes.append(slots)
        prog.append(lanes)

    import ml_dtypes
    offs, tot = [], 0
    for g in range(NG):
        offs.append(tot)
        tot += max(len(prog[g][a]) for a in range(4))
    wpk = np.zeros((128, tot * BLK), dtype=ml_dtypes.bfloat16)
    for g in range(NG):
        for a in range(4):
            for idx, (_, _, _, _, wt) in enumerate(prog[g][a]):
                col = (offs[g] + idx) * BLK
                wpk[32 * a:32 * a + 32, col:col + BLK] = wt.astype(
                    ml_dtypes.bfloat16)
    return prog, offs, tot, wpk, rmap


def kernel(x, w, rows, cols, out_blocks=None):
    import ml_dtypes
    import concourse.bass as bass
    import concourse.bacc as bacc
    import concourse.tile as tile
    import concourse.mybir as mybir
    from concourse.bass_utils import run_bass_kernel_spmd
    from contextlib import ExitStack

    x = np.asarray(x, dtype=np.float32)
    w = np.asarray(w, dtype=np.float32)
    rows = np.asarray(rows).astype(np.int64)
    cols = np.asarray(cols).astype(np.int64)

    prog, offs, tot, wpk, rmap = _build_schedule(w, rows, cols)

    # x^T, per-core partition-major: xarr[core, p, t*BC + j] = x[BC*core + j, 128*t + p]
    xarr = np.ascontiguousarray(
        x.reshape(NCORES, BC, NT, 128).transpose(0, 3, 2, 1)
    ).reshape(NCORES, 128, NT * BC).astype(ml_dtypes.bfloat16)

    f32 = mybir.dt.float32
    bf16 = mybir.dt.bfloat16
    nc = bacc.Bacc()
    xt_d = nc.declare_dram_parameter("xt", [128, NT * BC], bf16, isOutput=False)
    wp_d = nc.declare_dram_parameter("wpk", [128, tot * BLK], bf16, isOutput=False)
    yt_d = nc.declare_dram_parameter("yt", [128, NG, BC], f32, isOutput=True)

    with tile.TileContext(nc) as tc, ExitStack() as ctx:
        xp = ctx.enter_context(tc.tile_pool(name="x", bufs=1))
        wpool = ctx.enter_context(tc.tile_pool(name="w", bufs=4))
        pp = ctx.enter_context(tc.tile_pool(name="ps", bufs=8, space="PSUM"))
        tp = ctx.enter_context(tc.tile_pool(name="tmp", bufs=2))
        yp = ctx.enter_context(tc.tile_pool(name="y", bufs=4))

        wtiles = {}

        def load_w(g):
            n_g = max(len(prog[g][a]) for a in range(4))
            wsb = wpool.tile([128, n_g * BLK], bf16, tag="w", name=f"w{g}")
            nc.sync.dma_start(
                wsb[:], wp_d[:, offs[g] * BLK:(offs[g] + n_g) * BLK])
            wtiles[g] = wsb

        # x^T in 8 chunks interleaved with the first w groups: DMA rings
        # are FIFO, so early weights and early x chunks must lead the queue
        XC = NT // 8
        xts = []

        def load_x(ci):
            xc = xp.tile([128, XC * BC], bf16, tag=f"xc{ci}", name=f"xc{ci}")
            nc.sync.dma_start(
                xc[:], xt_d[:, ci * XC * BC:(ci + 1) * XC * BC])
            xts.append(xc)

        load_w(0)
        load_x(0)
        load_x(1)
        load_w(1)
        load_x(2)
        load_w(2)
        for ci in range(3, 8):
            load_x(ci)

        def rhs_of(t):
            return xts[t // XC][:, (t % XC) * BC:(t % XC + 1) * BC]

        for g in range(NG):
            if g + 3 < NG:
                load_w(g + 3)
            wsb = wtiles.pop(g)
            n_g = max(len(prog[g][a]) for a in range(4))
            ps = [pp.tile([128, BC], f32, tag="ps", name=f"ps{a}")
                  for a in range(4)]
            for idx in range(n_g):
                for a in range(4):
                    if idx < len(prog[g][a]):
                        t, b, st, sp, _ = prog[g][a][idx]
                        nc.tensor.matmul(
                            ps[a][32 * b:32 * b + 32, :],
                            lhsT=wsb[32 * a:32 * a + 32, idx * BLK:(idx + 1) * BLK],
                            rhs=rhs_of(t)[32 * a:32 * a + 32, :],
                            start=st, stop=sp,
                            tile_position=(32 * a, 32 * b),
                        )
            # PSUM has a single DVE read port: at most one PSUM operand per
            # DVE op. ACT evacuates two banks, DVE folds the rest.
            s0 = tp.tile([128, BC], f32, tag="t0")
            nc.scalar.copy(s0[:], ps[0][:])
            s2 = tp.tile([128, BC], f32, tag="t1")
            nc.scalar.copy(s2[:], ps[2][:])
            a01 = tp.tile([128, BC], f32, tag="t2")
            nc.vector.tensor_add(a01[:], s0[:], ps[1][:])
            a23 = tp.tile([128, BC], f32, tag="t3")
            nc.vector.tensor_add(a23[:], s2[:], ps[3][:])
            yt_t = yp.tile([128, BC], f32, tag="y")
            nc.vector.tensor_add(yt_t[:], a01[:], a23[:])
            nc.scalar.dma_start(yt_d[:, g, :], yt_t[:])

    nc.compile()

    in_maps = [{"xt": xarr[i], "wpk": wpk} for i in range(NCORES)]
    res = run_bass_kernel_spmd(
        nc, in_maps, list(range(NCORES)),
        trace=_RUN["trace"], trace_cores=_RUN["trace_cores"],
    )
    _RUN["last"] = res

    feat = np.empty(OUT, dtype=np.int64)
    for g in range(NG):
        for b in range(4):
            feat[128 * g + 32 * b:128 * g + 32 * b + 32] = \
                32 * rmap[g, b] + np.arange(32)

    y = np.empty((B, OUT), dtype=np.float32)
    for i in range(NCORES):
        ytp = res.results[i]["yt"].transpose(1, 0, 2).reshape(OUT, BC)
        yT = np.empty((OUT, BC), dtype=np.float32)
        yT[feat] = ytp
        y[BC * i:BC * (i + 1), :] = yT.T
    return y

